# revision 1
# baseline (speedup 1.0000x reference)
"""Fused full-attention kernel for Trainium2, SPMD over 8 NeuronCores.

Problem: nn_CausalSelfAttention (B=4, T=2048, D=1024, H=16, head_dim=64),
with the module's faithful-to-torch raw `.view(3,B,T,D)` reinterpretation of
the (B,T,3D) QKV projection buffer (NOT a feature-dim chunk), full (non-causal)
softmax over keys.

Sharding: core c handles batch b=c//2 and head-group hg=c%2 (8 heads). The raw
view means q/k/v token rows map to proj rows n//3 with column-chunk n%3; tokens
are processed in residue-class order (t mod 3), which makes every extraction a
contiguous slice. The host pre-permutes W_qkv columns per (b,hg,class) and
slices x rows per class, so one canonical SPMD program serves all cores. The
final output projection is computed per-core on the head-group's 512 columns;
host sums the two partial outputs per batch, un-permutes rows, and adds b_out.

NOTE: b_qkv is compiled in as zero (the problem spec fixes fill=zeros for it).
"""

import numpy as np

import concourse.mybir as mybir
from concourse import bacc
from concourse.bass_utils import run_bass_kernel_spmd
from concourse.tile import TileContext

F32 = mybir.dt.float32
F32R = mybir.dt.float32r
Exp = mybir.ActivationFunctionType.Exp

B, T, D = 4, 2048, 1024
CNT = (683, 683, 682)  # tokens per residue class (t % 3 == j)
OFF = (0, 683, 1366)

# t-tiles over the class-grouped token axis: (class j, in-class offset, rows)
TT = [
    (j, i0, min(128, CNT[j] - i0)) for j in range(3) for i0 in range(0, CNT[j], 128)
]
NTT = len(TT)  # 18


def build(reps: int = 1, g1_reps: int = 1, att_reps: int = 1):
    nc = bacc.Bacc("TRN2", target_bir_lowering=False, debug=False)

    xq = nc.dram_tensor("xq", (D, 2048), F32R, kind="ExternalInput")
    xk = nc.dram_tensor("xk", (D, 2048), F32R, kind="ExternalInput")
    xv = nc.dram_tensor("xv", (D, 2048), F32R, kind="ExternalInput")
    wq = nc.dram_tensor("wq", (D, 1536), F32R, kind="ExternalInput")
    wk = nc.dram_tensor("wk", (D, 1536), F32R, kind="ExternalInput")
    wv = nc.dram_tensor("wv", (D, 1536), F32R, kind="ExternalInput")
    wo = nc.dram_tensor("wo", (512, 1024), F32R, kind="ExternalInput")
    ones_d = nc.dram_tensor("ones_d", (128, 8), F32R, kind="ExternalInput")
    out = nc.dram_tensor("out", (2048, 1024), F32, kind="ExternalOutput")

    with TileContext(nc) as tc:
        with tc.tile_pool(name="pers", bufs=1) as pers:
            qTs = [pers.tile([128, 2048], F32R, tag=f"qT{i}", name=f"qT{i}") for i in range(4)]
            kTs = [pers.tile([128, 2048], F32R, tag=f"kT{i}", name=f"kT{i}") for i in range(4)]
            vs = [pers.tile([128, 520], F32R, tag=f"v{t}", name=f"v{t}") for t in range(NTT)]

            for _rep in range(reps):
              for _g1rep in range(g1_reps):
                # ---------------- GEMM1 ----------------
                with (
                      tc.tile_pool(name="g1x", bufs=12) as xp,
                      tc.tile_pool(name="g1w", bufs=3) as wp,
                      tc.tile_pool(name="g1wv", bufs=8) as wvp,
                      tc.tile_pool(name="g1ps", bufs=6, space="PSUM") as pp,
                  ):
                      # q and k: transposed-layout proj  [f, tok]
                      for xd, wd, dst in ((xq, wq, qTs), (xk, wk, kTs)):
                          wd_r = wd.rearrange("(dt p) c -> p dt c", p=128)
                          for j in range(3):
                              xts = []
                              for d in range(8):
                                  xt = xp.tile([128, 704], F32R, tag="x")
                                  nc.sync.dma_start(
                                      xt[:, 0 : CNT[j]],
                                      xd[d * 128 : (d + 1) * 128, OFF[j] : OFF[j] + CNT[j]],
                                  )
                                  xts.append(xt)
                              for fp in range(4):
                                  wt = wp.tile([128, 1024], F32R, tag="w")
                                  c0 = j * 512 + fp * 128
                                  nc.sync.dma_start(
                                      wt[:].rearrange("p (dt c) -> p dt c", c=128),
                                      wd_r[:, :, c0 : c0 + 128],
                                  )
                                  chunks = (
                                      ((0, 384), (CNT[j] - 300, 300))
                                      if CNT[j] % 2
                                      else ((0, 384), (384, CNT[j] - 384))
                                  )
                                  for a0, an in chunks:
                                      ps = pp.tile([128, 512], F32, tag="ps")
                                      for d in range(8):
                                          nc.tensor.matmul(
                                              ps[:, 0:an],
                                              wt[:, d * 128 : (d + 1) * 128],
                                              xts[d][:, a0 : a0 + an],
                                              start=(d == 0),
                                              stop=(d == 7),
                                          )
                                      nc.vector.tensor_copy(
                                          dst[fp][:, OFF[j] + a0 : OFF[j] + a0 + an],
                                          ps[:, 0:an],
                                      )
                      # v: natural layout [tok, f], interleaved with a ones column
                      for j in range(3):
                          xts = []
                          for d in range(8):
                              xt = xp.tile([128, 704], F32R, tag="x")
                              nc.sync.dma_start(
                                  xt[:, 0 : CNT[j]],
                                  xv[d * 128 : (d + 1) * 128, OFF[j] : OFF[j] + CNT[j]],
                              )
                              xts.append(xt)
                          wvts = []
                          for d in range(8):
                              wvt = wvp.tile([128, 512], F32R, tag="wv")
                              nc.sync.dma_start(
                                  wvt[:],
                                  wv[d * 128 : (d + 1) * 128, j * 512 : (j + 1) * 512],
                              )
                              wvts.append(wvt)
                          for tt, (jj, i0, tp) in enumerate(TT):
                              if jj != j:
                                  continue
                              ps = pp.tile([128, 512], F32, tag="ps")
                              for d in range(8):
                                  nc.tensor.matmul(
                                      ps[0:tp, :],
                                      xts[d][:, i0 : i0 + tp],
                                      wvts[d][:],
                                      start=(d == 0),
                                      stop=(d == 7),
                                  )
                              vr = vs[tt][0:tp, :].rearrange("p (h e) -> p h e", e=65)
                              nc.vector.tensor_copy(
                                  vr[:, :, 0:64],
                                  ps[0:tp, :].rearrange("p (h e) -> p h e", e=64),
                              )
                              nc.sync.dma_start(vr[:, :, 64:65], ones_d[0:tp, :])

              for _attrep in range(att_reps):
                # ---------------- attention ----------------
                with tc.tile_pool(name="att_pers", bufs=1) as apers:
                  inTs = [apers.tile([128, 2048], F32R, tag=f"inT{i}", name=f"inT{i}_{_rep}_{_attrep}") for i in range(4)]
                  wos = [apers.tile([128, 1024], F32R, tag=f"wo{i}", name=f"wo{i}_{_rep}_{_attrep}") for i in range(4)]
                  for i in range(4):
                      nc.sync.dma_start(wos[i][:], wo[i * 128 : (i + 1) * 128, :])
                  with (
                    tc.tile_pool(name="att_st", bufs=3, space="PSUM") as ap_st,
                    tc.tile_pool(name="att_in", bufs=1, space="PSUM") as ap_in,
                    tc.tile_pool(name="att_ex", bufs=8) as exp_,
                    tc.tile_pool(name="att_sm", bufs=2) as sm,
                  ):
                      for sblk in range(4):
                          for fp in range(4):
                              hA, hB = 2 * fp, 2 * fp + 1
                              sc0 = sblk * 512
                              inA = ap_in.tile([128, 512], F32, tag="inA")
                              inB = ap_in.tile([128, 512], F32, tag="inB")
                              for tt, (j, i0, tp) in enumerate(TT):
                                  t0 = OFF[j] + i0
                                  st = ap_st.tile([128, 1024], F32, tag="st")
                                  nc.tensor.matmul(
                                      st[0:tp, 0:512],
                                      kTs[fp][0:64, t0 : t0 + tp],
                                      qTs[fp][0:64, sc0 : sc0 + 512],
                                      start=True, stop=True, tile_position=(0, 0),
                                  )
                                  nc.tensor.matmul(
                                      st[0:tp, 512:1024],
                                      kTs[fp][64:128, t0 : t0 + tp],
                                      qTs[fp][64:128, sc0 : sc0 + 512],
                                      start=True, stop=True, tile_position=(64, 0),
                                  )
                                  ex = exp_.tile([128, 1024], F32R, tag="ex")
                                  nc.scalar.activation(
                                      ex[0:tp, :], st[0:tp, :], Exp, scale=0.125
                                  )
                                  nc.tensor.matmul(
                                      inA[0:65, :],
                                      vs[tt][0:tp, hA * 65 : hA * 65 + 65],
                                      ex[0:tp, 0:512],
                                      start=(tt == 0), stop=(tt == NTT - 1),
                                  )
                                  nc.tensor.matmul(
                                      inB[0:65, :],
                                      vs[tt][0:tp, hB * 65 : hB * 65 + 65],
                                      ex[0:tp, 512:1024],
                                      start=(tt == 0), stop=(tt == NTT - 1),
                                  )
                              recA = sm.tile([1, 512], F32, tag="rA")
                              recB = sm.tile([1, 512], F32, tag="rB")
                              nc.vector.reciprocal(recA[:], inA[64:65, :])
                              nc.vector.reciprocal(recB[:], inB[64:65, :])
                              bcA = sm.tile([64, 512], F32, tag="bA")
                              bcB = sm.tile([64, 512], F32, tag="bB")
                              nc.gpsimd.partition_broadcast(bcA[:], recA[:])
                              nc.gpsimd.partition_broadcast(bcB[:], recB[:])
                              nc.vector.tensor_mul(
                                  inTs[fp][0:64, sc0 : sc0 + 512], inA[0:64, :], bcA[:]
                              )
                              stB = sm.tile([64, 512], F32R, tag="sB")
                              nc.vector.tensor_mul(stB[:], inB[0:64, :], bcB[:])
                              nc.sync.dma_start(
                                  inTs[fp][64:128, sc0 : sc0 + 512], stB[:]
                              )

                  # ---------------- output projection ----------------
                  with (
                      tc.tile_pool(name="op_ps", bufs=4, space="PSUM") as opp,
                      tc.tile_pool(name="op_o", bufs=3) as obp,
                  ):
                      for s16 in range(16):
                          ot = obp.tile([128, 1024], F32, tag="ot")
                          for nb in range(2):
                              ps = opp.tile([128, 512], F32, tag="op")
                              for fp in range(4):
                                  nc.tensor.matmul(
                                      ps[:],
                                      inTs[fp][:, s16 * 128 : (s16 + 1) * 128],
                                      wos[fp][:, nb * 512 : (nb + 1) * 512],
                                      start=(fp == 0), stop=(fp == 3),
                                  )
                              nc.vector.tensor_copy(ot[:, nb * 512 : (nb + 1) * 512], ps[:])
                          nc.sync.dma_start(out[s16 * 128 : (s16 + 1) * 128, :], ot[:])

    nc.compile()
    return nc


_CACHE: dict = {}


def get_nc(reps: int = 1, g1_reps: int = 1, att_reps: int = 1):
    key = (reps, g1_reps, att_reps)
    if key not in _CACHE:
        _CACHE[key] = build(reps, g1_reps, att_reps)
    return _CACHE[key]


def shard_inputs(x, W_qkv, W_out):
    xf = np.ascontiguousarray(np.asarray(x, dtype=np.float32)).reshape(B * T, D)
    W_qkv = np.asarray(W_qkv, dtype=np.float32)
    W_out = np.asarray(W_out, dtype=np.float32)
    ones = np.ones((128, 8), np.float32)
    per_core = []
    for c in range(8):
        b, hg = c // 2, c % 2
        XQ = np.zeros((2048, D), np.float32)
        XK = np.zeros((2048, D), np.float32)
        XV = np.zeros((2048, D), np.float32)
        WQ = np.zeros((D, 1536), np.float32)
        WK = np.zeros((D, 1536), np.float32)
        WV = np.zeros((D, 1536), np.float32)
        for j in range(3):
            cnt, off = CNT[j], OFF[j]
            for XX, WW, base in (
                (XQ, WQ, b * 2048 + j),
                (XK, WK, 8192 + b * 2048 + j),
                (XV, WV, 16384 + b * 2048 + j),
            ):
                r0, ch = base // 3, base % 3
                XX[off : off + cnt] = xf[r0 : r0 + cnt]
                WW[:, j * 512 : (j + 1) * 512] = W_qkv[
                    :, ch * 1024 + hg * 512 : ch * 1024 + hg * 512 + 512
                ]
        per_core.append(
            dict(
                xq=np.ascontiguousarray(XQ.T),
                xk=np.ascontiguousarray(XK.T),
                xv=np.ascontiguousarray(XV.T),
                wq=WQ, wk=WK, wv=WV,
                wo=np.ascontiguousarray(W_out[hg * 512 : (hg + 1) * 512]),
                ones_d=ones,
            )
        )
    return per_core


_PI = np.concatenate([np.arange(j, 2048, 3) for j in range(3)])


def unshard(core_outs, b_out):
    b_out = np.asarray(b_out, dtype=np.float32)
    out = np.empty((B, T, D), np.float32)
    for b in range(B):
        part = core_outs[2 * b] + core_outs[2 * b + 1]
        tmp = np.empty_like(part)
        tmp[_PI] = part
        out[b] = tmp + b_out
    return out


def kernel(x, W_qkv, b_qkv, W_out, b_out, num_heads):
    assert int(num_heads) == 16
    nc = get_nc(1)
    in_maps = shard_inputs(x, W_qkv, W_out)
    res = run_bass_kernel_spmd(nc, in_maps, core_ids=list(range(8)))
    return unshard([r["out"] for r in res.results], b_out)



# revision 5
# speedup vs baseline: 2.7732x; 2.7732x over previous
"""Fused full-attention kernel for Trainium2, SPMD over 8 NeuronCores.

Problem: nn_CausalSelfAttention (B=4, T=2048, D=1024, H=16, head_dim=64),
with the module's faithful-to-torch raw `.view(3,B,T,D)` reinterpretation of
the (B,T,3D) QKV projection buffer (NOT a feature-dim chunk), full (non-causal)
softmax over keys.

Sharding: core c handles batch b=c//2 and head-group hg=c%2 (8 heads). The raw
view means q/k/v token rows map to proj rows n//3 with column-chunk n%3; tokens
are processed in residue-class order (t mod 3), which makes every extraction a
contiguous slice. The host pre-permutes W_qkv columns per (b,hg,class) and
slices x rows per class, so one canonical SPMD program serves all cores. The
final output projection is computed per-core on the head-group's 512 columns;
host sums the two partial outputs per batch, un-permutes rows, and adds b_out.

v2: all operands bf16 (validated ~3e-3 end-to-end error vs the 2e-2 gate),
uniform 16x128 token tiles for attention (v realigned across class boundaries
by partition-shifting DMAs), softmax-denominator via a ones-column interleaved
into v (accumulates in PSUM alongside the AV matmul), and interleaved emission:
the q-projection for later classes and the output-projection chunks are slotted
between attention segments so the PE fills the slack of the ACT(exp)-bound
attention phase.

NOTE: b_qkv is compiled in as zero (the problem spec fixes fill=zeros for it).
"""

import numpy as np
import ml_dtypes

import concourse.mybir as mybir
from concourse import bacc
from concourse.bass_utils import run_bass_kernel_spmd
from concourse.tile import TileContext

F32 = mybir.dt.float32
BF16 = mybir.dt.bfloat16
Exp = mybir.ActivationFunctionType.Exp

B, T, D = 4, 2048, 1024
CNT = (683, 683, 682)  # tokens per residue class (t % 3 == j)
OFF = (0, 683, 1366)
NG = 16  # uniform 128-token tiles over the grouped token axis


def build(reps: int = 1):
    nc = bacc.Bacc("TRN2", target_bir_lowering=False, debug=False)

    xq = nc.dram_tensor("xq", (D, 2048), BF16, kind="ExternalInput")
    xk = nc.dram_tensor("xk", (D, 2048), BF16, kind="ExternalInput")
    xv = nc.dram_tensor("xv", (D, 2048), BF16, kind="ExternalInput")
    wq = nc.dram_tensor("wq", (D, 1536), BF16, kind="ExternalInput")
    wk = nc.dram_tensor("wk", (D, 1536), BF16, kind="ExternalInput")
    wv = nc.dram_tensor("wv", (D, 1536), BF16, kind="ExternalInput")
    wo = nc.dram_tensor("wo", (512, 1024), BF16, kind="ExternalInput")
    out = nc.dram_tensor("out", (2048, 1024), BF16, kind="ExternalOutput")

    with TileContext(nc) as tc:
        with (
            tc.tile_pool(name="pers", bufs=1) as pers,
            tc.tile_pool(name="g1x", bufs=10) as xp,
            tc.tile_pool(name="g1w", bufs=3) as wp,
            tc.tile_pool(name="g1wv", bufs=8) as wvp,
            tc.tile_pool(name="mm", bufs=2, space="PSUM") as mmp,
            tc.tile_pool(name="att_st", bufs=2, space="PSUM") as stp,
            tc.tile_pool(name="att_in", bufs=1, space="PSUM") as inp,
            tc.tile_pool(name="att_ex", bufs=6) as exp_,
            tc.tile_pool(name="att_sm", bufs=3) as smp,
            tc.tile_pool(name="op_o", bufs=3) as otp,
        ):
            qTs = [pers.tile([128, 2048], BF16, tag=f"qT{i}", name=f"qT{i}") for i in range(4)]
            kTs = [pers.tile([128, 2048], BF16, tag=f"kT{i}", name=f"kT{i}") for i in range(4)]
            vs = [pers.tile([128, 520], BF16, tag=f"v{g}", name=f"v{g}") for g in range(NG)]
            inTs = [pers.tile([128, 2048], BF16, tag=f"inT{i}", name=f"inT{i}") for i in range(4)]
            wos = [pers.tile([128, 1024], BF16, tag=f"wo{i}", name=f"wo{i}") for i in range(4)]

            # ones columns of v (softmax denominator trick): written once,
            # disjoint from the per-rep data columns.
            for g in range(NG):
                vr = vs[g].rearrange("p (h e) -> p h e", e=65)
                nc.vector.memset(vr[:, :, 64:65], 1.0)

            def load_x(xd, j):
                xts = []
                for d in range(8):
                    xt = xp.tile([128, 704], BF16, tag="x")
                    nc.sync.dma_start(
                        xt[:, 0 : CNT[j]],
                        xd[d * 128 : (d + 1) * 128, OFF[j] : OFF[j] + CNT[j]],
                    )
                    xts.append(xt)
                return xts

            def qk_unit(xts, wd, dst, j, fp):
                # one (class j, head-pair fp) unit of the q/k projection:
                # transposed-layout [f, tok] via stationary weight chunks
                wd_r = wd.rearrange("(dt p) c -> p dt c", p=128)
                wt = wp.tile([128, 1024], BF16, tag="w")
                c0 = j * 512 + fp * 128
                nc.sync.dma_start(
                    wt[:].rearrange("p (dt c) -> p dt c", c=128),
                    wd_r[:, :, c0 : c0 + 128],
                )
                chunks = (
                    ((0, 384), (CNT[j] - 300, 300))
                    if CNT[j] % 2
                    else ((0, 384), (384, CNT[j] - 384))
                )
                for a0, an in chunks:
                    ps = mmp.tile([128, 512], F32, tag="mm")
                    for d in range(8):
                        nc.tensor.matmul(
                            ps[:, 0:an],
                            wt[:, d * 128 : (d + 1) * 128],
                            xts[d][:, a0 : a0 + an],
                            start=(d == 0),
                            stop=(d == 7),
                        )
                    nc.vector.tensor_copy(
                        dst[fp][:, OFF[j] + a0 : OFF[j] + a0 + an],
                        ps[:, 0:an],
                    )

            def emit_k():
                for j in range(3):
                    xts = load_x(xk, j)
                    for fp in range(4):
                        qk_unit(xts, wk, kTs, j, fp)

            def emit_v():
                # natural layout [tok, f]; realigned into uniform 128-token
                # vs tiles (interleaved 65-stride with the ones columns) by
                # partition-shifting DMAs.
                for j in range(3):
                    xts = load_x(xv, j)
                    wvts = []
                    for d in range(8):
                        wvt = wvp.tile([128, 512], BF16, tag="wv")
                        nc.sync.dma_start(
                            wvt[:],
                            wv[d * 128 : (d + 1) * 128, j * 512 : (j + 1) * 512],
                        )
                        wvts.append(wvt)
                    for i0 in range(0, CNT[j], 128):
                        tp = min(128, CNT[j] - i0)
                        ps = mmp.tile([128, 512], F32, tag="mm")
                        for d in range(8):
                            nc.tensor.matmul(
                                ps[0:tp, :],
                                xts[d][:, i0 : i0 + tp],
                                wvts[d][:],
                                start=(d == 0),
                                stop=(d == 7),
                            )
                        vst = wvp.tile([128, 512], BF16, tag="vst")
                        nc.vector.tensor_copy(vst[0:tp, :], ps[0:tp, :])
                        vsr = vst.rearrange("p (h e) -> p h e", e=64)
                        g0 = OFF[j] + i0
                        g, p0 = g0 // 128, g0 % 128
                        r1 = min(tp, 128 - p0)
                        vr = vs[g].rearrange("p (h e) -> p h e", e=65)
                        nc.sync.dma_start(
                            vr[p0 : p0 + r1, :, 0:64], vsr[0:r1, :, :]
                        )
                        if tp > r1:
                            vr2 = vs[g + 1].rearrange("p (h e) -> p h e", e=65)
                            nc.sync.dma_start(
                                vr2[0 : tp - r1, :, 0:64], vsr[r1:tp, :, :]
                            )

            def q_units(j):
                # returns 4 closures (one per fp) emitting q-projection pieces
                state = {}

                def unit(fp):
                    if "xts" not in state:
                        state["xts"] = load_x(xq, j)
                    qk_unit(state["xts"], wq, qTs, j, fp)

                return [lambda fp=fp: unit(fp) for fp in range(4)]

            def load_wo():
                for i in range(4):
                    nc.sync.dma_start(wos[i][:], wo[i * 128 : (i + 1) * 128, :])

            def attn_segment(s, fp):
                sc0 = s * 512
                hA, hB = 2 * fp, 2 * fp + 1
                inA = inp.tile([65, 512], F32, tag="inA")
                inB = inp.tile([65, 512], F32, tag="inB")
                for g in range(NG):
                    t0 = g * 128
                    st = stp.tile([128, 1024], F32, tag="st")
                    nc.tensor.matmul(
                        st[:, 0:512],
                        kTs[fp][0:64, t0 : t0 + 128],
                        qTs[fp][0:64, sc0 : sc0 + 512],
                        start=True, stop=True, tile_position=(0, 0),
                    )
                    nc.tensor.matmul(
                        st[:, 512:1024],
                        kTs[fp][64:128, t0 : t0 + 128],
                        qTs[fp][64:128, sc0 : sc0 + 512],
                        start=True, stop=True, tile_position=(64, 0),
                    )
                    ex = exp_.tile([128, 1024], BF16, tag="ex")
                    nc.scalar.activation(ex[:], st[:], Exp, scale=0.125)
                    nc.tensor.matmul(
                        inA[:],
                        vs[g][:, hA * 65 : hA * 65 + 65],
                        ex[:, 0:512],
                        start=(g == 0), stop=(g == NG - 1),
                    )
                    nc.tensor.matmul(
                        inB[:],
                        vs[g][:, hB * 65 : hB * 65 + 65],
                        ex[:, 512:1024],
                        start=(g == 0), stop=(g == NG - 1),
                    )
                # copy PSUM accumulators to SBUF promptly (frees the banks for
                # the next segment), then normalize by the ones-row sums.
                sAB = smp.tile([65, 1024], F32, tag="sAB")
                nc.vector.tensor_copy(sAB[:, 0:512], inA[:])
                nc.vector.tensor_copy(sAB[:, 512:1024], inB[:])
                rec = smp.tile([1, 1024], F32, tag="rec")
                nc.vector.reciprocal(rec[:], sAB[64:65, :])
                bc = smp.tile([64, 1024], F32, tag="bc")
                nc.gpsimd.partition_broadcast(bc[:], rec[:])
                nc.vector.tensor_mul(
                    inTs[fp][0:64, sc0 : sc0 + 512], sAB[0:64, 0:512], bc[:, 0:512]
                )
                stB = smp.tile([64, 512], BF16, tag="stB")
                nc.vector.tensor_mul(stB[:], sAB[0:64, 512:1024], bc[:, 512:1024])
                nc.sync.dma_start(inTs[fp][64:128, sc0 : sc0 + 512], stB[:])

            def op_chunk(s16):
                ot = otp.tile([128, 1024], BF16, tag="ot")
                for nb in range(2):
                    ps = mmp.tile([128, 512], F32, tag="mm")
                    for fp in range(4):
                        nc.tensor.matmul(
                            ps[:],
                            inTs[fp][:, s16 * 128 : (s16 + 1) * 128],
                            wos[fp][:, nb * 512 : (nb + 1) * 512],
                            start=(fp == 0), stop=(fp == 3),
                        )
                    nc.vector.tensor_copy(ot[:, nb * 512 : (nb + 1) * 512], ps[:])
                nc.sync.dma_start(out[s16 * 128 : (s16 + 1) * 128, :], ot[:])

            for _rep in range(reps):
                load_wo()
                emit_k()
                emit_v()
                for u in q_units(0):
                    u()
                # attention sblk s needs q columns [512s, 512s+512), available
                # after q classes covering that grouped range. Interleave the
                # remaining q classes and the out-projection chunks into the
                # ACT-bound attention segments.
                extras = {
                    0: q_units(1),
                    1: q_units(2),
                    2: [lambda i=i: op_chunk(0 * 4 + i) for i in range(4)],
                    3: [
                        lambda i=i: (op_chunk(1 * 4 + i), op_chunk(2 * 4 + i))
                        for i in range(4)
                    ],
                }
                for s in range(4):
                    ex_s = extras[s]
                    for fp in range(4):
                        attn_segment(s, fp)
                        ex_s[fp]()
                for i in range(4):
                    op_chunk(3 * 4 + i)

    nc.compile()
    return nc


_CACHE: dict = {}


def get_nc(reps: int = 1):
    if reps not in _CACHE:
        _CACHE[reps] = build(reps)
    return _CACHE[reps]


def shard_inputs(x, W_qkv, W_out):
    bf16 = ml_dtypes.bfloat16
    xf = np.ascontiguousarray(np.asarray(x, dtype=np.float32)).reshape(B * T, D)
    W_qkv = np.asarray(W_qkv, dtype=np.float32)
    W_out = np.asarray(W_out, dtype=np.float32)
    per_core = []
    for c in range(8):
        b, hg = c // 2, c % 2
        XQ = np.zeros((2048, D), np.float32)
        XK = np.zeros((2048, D), np.float32)
        XV = np.zeros((2048, D), np.float32)
        WQ = np.zeros((D, 1536), np.float32)
        WK = np.zeros((D, 1536), np.float32)
        WV = np.zeros((D, 1536), np.float32)
        for j in range(3):
            cnt, off = CNT[j], OFF[j]
            for XX, WW, base in (
                (XQ, WQ, b * 2048 + j),
                (XK, WK, 8192 + b * 2048 + j),
                (XV, WV, 16384 + b * 2048 + j),
            ):
                r0, ch = base // 3, base % 3
                XX[off : off + cnt] = xf[r0 : r0 + cnt]
                WW[:, j * 512 : (j + 1) * 512] = W_qkv[
                    :, ch * 1024 + hg * 512 : ch * 1024 + hg * 512 + 512
                ]
        per_core.append(
            dict(
                xq=np.ascontiguousarray(XQ.T).astype(bf16),
                xk=np.ascontiguousarray(XK.T).astype(bf16),
                xv=np.ascontiguousarray(XV.T).astype(bf16),
                wq=WQ.astype(bf16), wk=WK.astype(bf16), wv=WV.astype(bf16),
                wo=np.ascontiguousarray(W_out[hg * 512 : (hg + 1) * 512]).astype(
                    bf16
                ),
            )
        )
    return per_core


_PI = np.concatenate([np.arange(j, 2048, 3) for j in range(3)])


def unshard(core_outs, b_out):
    b_out = np.asarray(b_out, dtype=np.float32)
    out = np.empty((B, T, D), np.float32)
    for b in range(B):
        part = np.asarray(core_outs[2 * b], np.float32) + np.asarray(
            core_outs[2 * b + 1], np.float32
        )
        tmp = np.empty_like(part)
        tmp[_PI] = part
        out[b] = tmp + b_out
    return out


def kernel(x, W_qkv, b_qkv, W_out, b_out, num_heads):
    assert int(num_heads) == 16
    nc = get_nc(1)
    in_maps = shard_inputs(x, W_qkv, W_out)
    res = run_bass_kernel_spmd(nc, in_maps, core_ids=list(range(8)))
    return unshard([r["out"] for r in res.results], b_out)
